# revision 20
# baseline (speedup 1.0000x reference)
"""GQA causal attention (ternary weights) on 8 TRN2 NeuronCores.

Strategy (tensor-parallel over heads, per sharding hint):
  - core c owns Q heads [4c, 4c+4) and KV head c.
  - host: ternarize weights, transpose + fp16-hi/lo-split x (exact 2^10
    compensation so fp16 subnormal FTZ cannot hurt), slice weights per core.
  - device per core:
      phase 1: q/k/v projections as 2-pass fp16 matmuls (ternary weights are
               exact in fp16; lo pass uses 2^10-scaled x residual against
               2^-10-scaled weights).
      phase 2: per (batch, head): cheap 1-pass fp16 S=QK^T in [q,k] layout for
               a row-max estimate (software-pipelined one head ahead); then
               exact-enough S^T in [k,q] layout via two matmuls (hi pass with
               folded -max bias row, compensated residual pass), exp on
               ScalarE, and PV + row-sums via a single fp32r matmul with a
               ones column appended to V.
      phase 3: o_proj partial (this core's 256 input dims) via fp32r,
               emitted per batch so its DMA overlaps the other batch's
               attention.
  - host: sum the 8 partial outputs (the "all-reduce" of the row-split o_proj).
"""

import sys

sys.path.insert(0, "/opt/trn_rl_repo")

import numpy as np

B = 2
S = 2048
D = 2048
NCORES = 8
HEADS_PER_CORE = 4
HD = 64
QROWS = HEADS_PER_CORE * HD  # 256
TT = 512  # token tile
LO_SCALE = 1024.0  # 2**10 subnormal-avoidance scale for fp16 lo pieces
MASK_NEG = -30000.0

_CACHE = {}


def _build_program(b=B, s=S, d=D):
    import concourse.bacc as bacc
    import concourse.tile as tile
    import concourse.mybir as mybir
    from concourse import masks
    from contextlib import ExitStack

    f32 = mybir.dt.float32
    f32r = mybir.dt.float32r
    f16 = mybir.dt.float16
    Alu = mybir.AluOpType
    Act = mybir.ActivationFunctionType

    tokens = b * s
    n_tt = tokens // TT          # token tiles
    tt_per_b = s // TT
    n_dc = d // 128              # contraction chunks for projections
    n_qt = s // TT               # 512-wide q tiles per batch
    n_qc = s // 128              # 128-wide q chunks per batch (max pass)
    n_mt = d // 128              # output row tiles for o_proj
    n_oc = QROWS // 128          # o_proj contraction chunks (2)
    sub = TT // 128              # 128-sub-blocks per 512 tile (4)

    nc = bacc.Bacc("TRN2", target_bir_lowering=False, debug=False,
                   num_devices=NCORES)

    xh_d = nc.dram_tensor("xh", [d, tokens], f16, kind="ExternalInput").ap()
    xl_d = nc.dram_tensor("xl", [d, tokens], f16, kind="ExternalInput").ap()
    wqh_d = nc.dram_tensor("wq_hi", [d, QROWS], f16, kind="ExternalInput").ap()
    wql_d = nc.dram_tensor("wq_lo", [d, QROWS], f16, kind="ExternalInput").ap()
    wkh_d = nc.dram_tensor("wkv_hi", [d, 128], f16, kind="ExternalInput").ap()
    wkl_d = nc.dram_tensor("wkv_lo", [d, 128], f16, kind="ExternalInput").ap()
    wo_d = nc.dram_tensor("wo", [QROWS, d], f32r, kind="ExternalInput").ap()
    out_d = nc.dram_tensor("out", [d, tokens], f32, kind="ExternalOutput").ap()

    with tile.TileContext(nc) as tc, ExitStack() as top:
        constp = top.enter_context(tc.tile_pool(name="const", bufs=1))
        wpool = top.enter_context(tc.tile_pool(name="wts", bufs=1))
        pp = top.enter_context(tc.tile_pool(name="persist", bufs=1))

        # --- constants -------------------------------------------------
        maskM = constp.tile([128, 128], f32, tag="maskM")   # [k,q] diag: keep k<=q
        nc.gpsimd.memset(maskM[:], 0.0)
        nc.gpsimd.affine_select(
            out=maskM[:], in_=maskM[:], compare_op=Alu.is_ge, fill=MASK_NEG,
            base=0, pattern=[[1, 128]], channel_multiplier=-1)
        maskM2 = constp.tile([128, 128], f32, tag="maskM2")  # [q,k] diag: keep k<=q
        nc.gpsimd.memset(maskM2[:], 0.0)
        nc.gpsimd.affine_select(
            out=maskM2[:], in_=maskM2[:], compare_op=Alu.is_ge, fill=MASK_NEG,
            base=0, pattern=[[-1, 128]], channel_multiplier=1)
        ident = constp.tile([128, 128], f32, tag="ident")
        masks.make_identity(nc, ident[:])
        onesc = constp.tile([65, HD], f32r, tag="onesc")
        nc.scalar.activation(onesc[:], maskM[0:65, 0:HD], Act.Identity,
                             bias=1.0, scale=0.0)

        # --- weights ---------------------------------------------------
        wq_sb = {}
        for name, dram in (("hi", wqh_d), ("lo", wql_d)):
            t = wpool.tile([128, n_dc * QROWS], f16, tag=f"wq{name}",
                           name=f"wq{name}")
            nc.sync.dma_start(
                out=t[:].rearrange("p (c n) -> p c n", n=QROWS),
                in_=dram.rearrange("(c p) n -> p c n", p=128))
            wq_sb[name] = t
        wkv_sb = {}
        for name, dram in (("hi", wkh_d), ("lo", wkl_d)):
            t = wpool.tile([128, n_dc * 128], f16, tag=f"wkv{name}",
                           name=f"wkv{name}")
            nc.sync.dma_start(
                out=t[:].rearrange("p (c n) -> p c n", n=128),
                in_=dram.rearrange("(c p) n -> p c n", p=128))
            wkv_sb[name] = t

        # --- persistent activations -----------------------------------
        # qA[h]: rows 0:64 = fp16(q/8) "qh", row 64 = m~ bias (max pass)
        # qB[h]: rows 0:64 = qh * 2^-10, rows 64:128 = fp16(2^10 * (q/8 - qh))
        # khb:   rows 0:64 = fp16(k) "kh", row 64 = -1
        # klkh:  rows 0:64 = fp16(2^10 * (k - kh)), rows 64:128 = kh * 2^-10
        # vhat:  [128, chunk*65]: cols 0:64 of chunk = v (natural layout),
        #        col 64 = 1.0
        qA = [pp.tile([65, tokens], f16, tag=f"qA{h}", name=f"qA{h}")
              for h in range(HEADS_PER_CORE)]
        qB = [pp.tile([128, tokens], f16, tag=f"qB{h}", name=f"qB{h}")
              for h in range(HEADS_PER_CORE)]
        khb = pp.tile([65, tokens], f16, tag="khb")
        klkh = pp.tile([128, tokens], f16, tag="klkh")
        n_ch = tokens // 128
        vhat = pp.tile([128, n_ch * 65], f32r, tag="vhat")
        nc.scalar.activation(
            vhat[:], maskM[:, 0:1].to_broadcast([128, n_ch * 65]),
            Act.Identity, bias=1.0, scale=0.0)
        nc.gpsimd.memset(khb[64:65, :], -1.0)

        with ExitStack() as ph:
            mp = ph.enter_context(tc.tile_pool(name="mp", bufs=2))
            ps1 = ph.enter_context(
                tc.tile_pool(name="ps1", bufs=3, space="PSUM"))
            psst = ph.enter_context(
                tc.tile_pool(name="psst", bufs=2, space="PSUM"))
            psav = ph.enter_context(
                tc.tile_pool(name="psav", bufs=2, space="PSUM"))
            psbc = ph.enter_context(
                tc.tile_pool(name="psbc", bufs=1, space="PSUM"))

            # ---------- S~ max-estimate pass, as schedulable blocks ------
            mstate = {}
            mbp = ph.enter_context(tc.tile_pool(name="mbp", bufs=8))

            def s_block(bb, h, qc):
                boff = bb * s
                if qc == 0:
                    mstate[(bb, h)] = mbp.tile([128, n_qc], f32, tag="mbuf",
                                               name="mbuf")
                mbuf = mstate[(bb, h)]
                qsl = slice(boff + qc * 128, boff + qc * 128 + 128)
                ntk = qc // sub + 1
                mtmp = mp.tile([128, 8], f32, tag="mtmp")
                for kt in range(ntk):
                    w = min(TT, (qc + 1) * 128 - kt * TT)
                    st = psst.tile([128, TT], f32, tag="st")
                    nc.tensor.matmul(
                        st[:, 0:w],
                        lhsT=qA[h][0:64, qsl],
                        rhs=khb[0:64, boff + kt * TT:boff + kt * TT + w],
                        start=True, stop=True)
                    if kt == ntk - 1:  # diagonal block is last 128 cols
                        nc.vector.tensor_tensor(
                            st[:, w - 128:w], st[:, w - 128:w],
                            maskM2[:], op=Alu.add)
                    nc.vector.tensor_reduce(
                        mtmp[:, kt:kt + 1], st[:, 0:w],
                        axis=mybir.AxisListType.X, op=Alu.max)
                nc.vector.tensor_reduce(
                    mbuf[:, qc:qc + 1], mtmp[:, 0:ntk],
                    axis=mybir.AxisListType.X, op=Alu.max)

            def s_final(bb, h):
                boff = bb * s
                mbuf = mstate.pop((bb, h))
                mps = psst.tile([128, TT], f32, tag="st")
                nc.tensor.transpose(mps[0:n_qc, 0:128], mbuf[:, 0:n_qc],
                                    ident[:, 0:128])
                mrow = mp.tile([n_qc, 128], f32, tag="mrow")
                nc.vector.tensor_copy(mrow[:], mps[0:n_qc, 0:128])
                nc.gpsimd.dma_start(
                    out=qA[h][64:65, boff:boff + s].rearrange(
                        "o (c t) -> o c t", t=128),
                    in_=mrow[:])


            # ================= phase 1: projections ====================
            ph1 = ExitStack()
            xp = ph1.enter_context(tc.tile_pool(name="xp", bufs=3))
            sp1 = ph1.enter_context(tc.tile_pool(name="sp1", bufs=3))
            for tt in range(n_tt):
                tcols = slice(tt * TT, (tt + 1) * TT)
                x_sb = {}
                for name, dram in (("hi", xh_d), ("lo", xl_d)):
                    t = xp.tile([128, n_dc * TT], f16, tag="x", name="xtile")
                    nc.sync.dma_start(
                        out=t[:].rearrange("p (c t) -> p c t", t=TT),
                        in_=dram.rearrange("(c p) t -> p c t",
                                           p=128)[:, :, tcols])
                    x_sb[name] = t

                def proj(w_sb, mcol, mwid, ps):
                    first = True
                    for name in ("hi", "lo"):
                        for c in range(n_dc):
                            nc.tensor.matmul(
                                ps[:],
                                lhsT=w_sb[name][:, c * mwid + mcol:
                                                c * mwid + mcol + 128],
                                rhs=x_sb[name][:, c * TT:(c + 1) * TT],
                                start=first,
                                stop=(name == "lo" and c == n_dc - 1))
                            first = False

                for m in range(QROWS // 128):
                    ps = ps1.tile([128, TT], f32, tag="ps")
                    proj(wq_sb, m * 128, QROWS, ps)
                    tmp = sp1.tile([128, TT], f32, tag="qtmp")
                    nc.scalar.mul(tmp[:], ps[:], 0.125)
                    qh16 = sp1.tile([128, TT], f16, tag="qh16")
                    nc.scalar.copy(qh16[:], tmp[:])
                    res = sp1.tile([128, TT], f32, tag="qres")
                    nc.vector.tensor_tensor(
                        res[:], tmp[:], qh16[:], op=Alu.subtract)
                    for i in range(2):
                        h = 2 * m + i
                        rows = slice(i * 64, i * 64 + 64)
                        nc.scalar.copy(qA[h][0:64, tcols], qh16[rows, :])
                        nc.scalar.mul(qB[h][0:64, tcols], qh16[rows, :],
                                      1.0 / LO_SCALE)
                        nc.scalar.mul(qB[h][64:128, tcols], res[rows, :],
                                      LO_SCALE)

                ps = ps1.tile([128, TT], f32, tag="ps")
                proj(wkv_sb, 0, 128, ps)
                nc.scalar.copy(khb[0:64, tcols], ps[0:64, :])
                nc.scalar.mul(klkh[64:128, tcols], ps[0:64, :], 1.0 / LO_SCALE)
                res = sp1.tile([128, TT], f32, tag="qres")
                nc.vector.tensor_tensor(
                    res[0:64, :], ps[0:64, :], khb[0:64, tcols],
                    op=Alu.subtract)
                nc.scalar.mul(klkh[0:64, tcols], res[0:64, :], LO_SCALE)
                vtmp = sp1.tile([64, TT], f32, tag="vtmp")
                nc.scalar.copy(vtmp[:], ps[64:128, :])
                for j in range(sub):
                    ptr = psst.tile([128, TT], f32, tag="st")
                    nc.tensor.transpose(ptr[0:128, 0:64],
                                        vtmp[:, j * 128:(j + 1) * 128],
                                        ident[0:64, 0:64])
                    ch = tt * sub + j
                    nc.scalar.copy(vhat[:, ch * 65:ch * 65 + 64],
                                   ptr[0:128, 0:64])
                # S~ blocks for the q-chunks this tile just made available
                bb, ltt = tt // tt_per_b, tt % tt_per_b
                for h in range(HEADS_PER_CORE):
                    for qc in range(ltt * sub, (ltt + 1) * sub):
                        s_block(bb, h, qc)
                if ltt == tt_per_b - 1:
                    for h in range(HEADS_PER_CORE):
                        s_final(bb, h)

            ph1.close()
            # ============ phase 2 + per-batch o_proj ====================
            aop = ph.enter_context(tc.tile_pool(name="aop", bufs=1))
            ptp = ph.enter_context(tc.tile_pool(name="ptp", bufs=4))
            outp = ph.enter_context(tc.tile_pool(name="outp", bufs=2))
            ao = [aop.tile([128, tokens], f32r, tag=f"ao{i}", name=f"ao{i}")
                  for i in range(2)]
            wo_sb = aop.tile([128, n_oc * d], f32r, tag="wo")
            nc.sync.dma_start(
                out=wo_sb[:].rearrange("p (c n) -> p c n", n=d),
                in_=wo_d.rearrange("(c p) n -> p c n", p=128))

            def av(pav, pt, lo, w, bb, kc, nchunks):
                ch = bb * (s // 128) + kc
                nc.tensor.matmul(
                    pav[:, lo:lo + w], lhsT=vhat[:, ch * 65:ch * 65 + 65],
                    rhs=pt[:, lo:lo + w],
                    start=(kc == 0), stop=(kc == nchunks - 1),
                    skip_group_check=True)


            def oproj_m(bb, m):
                boff = bb * s
                osb = outp.tile([128, s], f32, tag="ot")
                for tt2 in range(tt_per_b):
                    po = ps1.tile([128, TT], f32, tag="ps")
                    for ci in range(n_oc):
                        nc.tensor.matmul(
                            po[:],
                            lhsT=wo_sb[:, ci * d + m * 128:
                                       ci * d + m * 128 + 128],
                            rhs=ao[ci][:, boff + tt2 * TT:
                                       boff + (tt2 + 1) * TT],
                            start=(ci == 0), stop=(ci == n_oc - 1))
                    nc.vector.tensor_copy(osb[:, tt2 * TT:(tt2 + 1) * TT],
                                          po[:])
                nc.sync.dma_start(
                    out=out_d[m * 128:(m + 1) * 128, boff:boff + s],
                    in_=osb[:])

            slot = 0

            for bb in range(b):
                boff = bb * s
                for h in range(HEADS_PER_CORE):
                    for qt in range(n_qt):
                        qlo = boff + qt * TT
                        pav = psav.tile([65, TT], f32, tag="pav")
                        nchunks = (qt + 1) * sub
                        prev = None
                        for kc in range(nchunks):
                            ksl = slice(boff + kc * 128, boff + kc * 128 + 128)
                            j = kc - qt * sub
                            lo = max(j, 0) * 128  # cols < lo fully masked
                            w = TT - lo
                            s2 = ps1.tile([128, TT], f32, tag="ps")
                            nc.tensor.matmul(
                                s2[:, lo:lo + w], lhsT=khb[:, ksl],
                                rhs=qA[h][:, qlo + lo:qlo + TT],
                                start=True, stop=False)
                            nc.tensor.matmul(
                                s2[:, lo:lo + w], lhsT=klkh[:, ksl],
                                rhs=qB[h][:, qlo + lo:qlo + TT],
                                start=False, stop=True)
                            if j >= 0:
                                nc.vector.tensor_tensor(
                                    s2[:, lo:lo + 128], s2[:, lo:lo + 128],
                                    maskM[:], op=Alu.add)
                            pt = ptp.tile([128, TT], f32r, tag="pt")
                            nc.scalar.activation(pt[:, lo:lo + w],
                                                 s2[:, lo:lo + w], Act.Exp)
                            if prev is not None:
                                av(pav, prev[0], prev[1], prev[2], bb,
                                   prev[3], nchunks)
                            prev = (pt, lo, w, kc)
                        av(pav, prev[0], prev[1], prev[2], bb, prev[3],
                           nchunks)

                        rec = mp.tile([65, TT], f32r, tag="rec")
                        with nc.allow_low_precision(
                                reason="1/l broadcast feeds fp32r matmul"):
                            nc.vector.reciprocal(rec[64:65, :],
                                                 pav[64:65, :])
                        bc = psbc.tile([64, TT], f32, tag="bc")
                        nc.tensor.matmul(
                            bc[:], lhsT=onesc[64:65, 0:HD],
                            rhs=rec[64:65, :], start=True, stop=True)
                        bcs = mp.tile([64, TT], f32, tag="bcs")
                        nc.scalar.copy(bcs[:], bc[:])
                        rows = slice((h % 2) * 64, (h % 2) * 64 + 64)
                        nc.vector.tensor_tensor(
                            ao[h // 2][rows, qlo:qlo + TT], pav[0:64, :],
                            bcs[:], op=Alu.mult)
                        # o_proj of the previous batch rides along here
                        if bb == 1:
                            nsl = HEADS_PER_CORE * n_qt
                            per = (n_mt + nsl - 1) // nsl
                            for m in range(slot * per,
                                           min((slot + 1) * per, n_mt)):
                                oproj_m(0, m)
                            slot += 1
            for m in range(n_mt):
                oproj_m(1, m)

    nc.compile()
    return nc


def _ternarize(w):
    w = np.asarray(w, np.float32)
    scale = max(np.abs(w).mean(), 1e-6)
    return ((w > 0.05 * scale).astype(np.float32)
            - (w < -0.05 * scale).astype(np.float32))


def _split_f16(a):
    hi = a.astype(np.float16)
    lo = ((a - hi.astype(np.float32)) * LO_SCALE).astype(np.float16)
    return hi, lo


def kernel(x, wq, wk, wv, wo):
    from concourse.bass_utils import run_bass_kernel_spmd

    if "nc" not in _CACHE:
        _CACHE["nc"] = _build_program()
    nc = _CACHE["nc"]

    tq = _ternarize(wq)
    tk = _ternarize(wk)
    tv = _ternarize(wv)
    to = _ternarize(wo)

    xT = np.ascontiguousarray(np.asarray(x, np.float32).reshape(B * S, D).T)
    xh, xl = _split_f16(xT)

    in_maps = []
    for c in range(NCORES):
        qsl = slice(c * QROWS, (c + 1) * QROWS)
        ksl = slice(c * HD, (c + 1) * HD)
        wkv = np.concatenate([tk[ksl], tv[ksl]], axis=0)  # [128, D]
        in_maps.append({
            "xh": xh, "xl": xl,
            "wq_hi": np.ascontiguousarray(tq[qsl].T).astype(np.float16),
            "wq_lo": np.ascontiguousarray(tq[qsl].T / LO_SCALE).astype(np.float16),
            "wkv_hi": np.ascontiguousarray(wkv.T).astype(np.float16),
            "wkv_lo": np.ascontiguousarray(wkv.T / LO_SCALE).astype(np.float16),
            "wo": np.ascontiguousarray(to[:, qsl].T).astype(np.float32),
        })

    res = run_bass_kernel_spmd(nc, in_maps, list(range(NCORES)))
    total = res.results[0]["out"]
    for c in range(1, NCORES):
        total = total + res.results[c]["out"]
    return np.ascontiguousarray(total.T).reshape(B, S, D).astype(np.float32)
